# revision 1
# baseline (speedup 1.0000x reference)
"""TRN2 Bass kernel for nn_LinearLoopLayer: out = x @ weights.T + bias.

Shapes (hardcoded): x [4096, 4096] f32, weights [4096, 4096] f32,
bias [4096] f32 -> out [4096, 4096] f32.

Strategy
--------
* Sharding: 2-way over batch x 4-way over out_features across the 8
  NeuronCores. Per core: x-shard [2048, 4096], W-shard [1024, 4096],
  bias-shard [1024] -> out-shard [2048, 1024]. This minimizes per-core
  HBM traffic (56.5MB/core vs 80MB for 1-D sharding) and keeps the
  W-shard SBUF-resident (16MB).
* Host-side prep (part of sharding): both matmul operands need the
  contraction dim (in_features) on SBUF partitions, so the shards are
  passed transposed (xT [4096, 2048], wT [4096, 1024]); fp32 has no DMA
  transpose path on TRN2.
* Matmul dtype float32r: PE streams it at 1 row/cycle (vs 4 for plain
  fp32) when the moving free dim >= 256, keeping ~11 mantissa bits.
  Overall rel error ~1e-4 (fp32 accumulation in PSUM).
* Per core: 1024 matmuls (lhsT = xT tile [128i, 128b] stationary,
  rhs = wT tile [128i, 512o] moving, PSUM [128b, 512o] accumulates over
  32 k-tiles). W streams in 4x4MB chunks on the SP HWDGE ring while x
  tiles ride the ACT HWDGE ring in paired 4MB loads (fewer DMAs = lower
  fixed costs; HW-measured ~30us faster than 8x2MB W + 2MB x). During
  the fill the first 4 m-tiles are striped by W k-chunk (8 PSUM banks)
  so the PE consumes each chunk as it lands. Bias is added during the
  PSUM->SBUF drain (DVE).
"""
import numpy as np

import concourse.bass as bass
import concourse.tile as tile
import concourse.mybir as mybir
from concourse import bacc
from concourse.bass_utils import run_bass_kernel_spmd

P = 128

BATCH = 4096
IN_F = 4096
OUT_F = 4096

B_SHARDS = 2
O_SHARDS = 4
N_CORES = 8

B_C = BATCH // B_SHARDS       # 2048 batch rows per core
O_C = OUT_F // O_SHARDS       # 1024 out features per core
KT = IN_F // P                # 32 k-tiles
MT = B_C // P                 # 16 m-tiles
NFREE = 512                   # moving free dim per matmul
NT = O_C // NFREE             # 2 n-tiles per m-tile

DT_MM = mybir.dt.float32r
DT_F32 = mybir.dt.float32

W_CHUNKS = 4                  # W streamed in 4 x 4MB chunks (fewer DMA fixed costs)
HEAD_M = 4                    # m-tiles in the fill-phase wavefront


def _build_kernel():
    nc = bacc.Bacc("TRN2", debug=False)

    xT = nc.dram_tensor("xT", [IN_F, B_C], DT_MM, kind="ExternalInput").ap()
    wT = nc.dram_tensor("wT", [IN_F, O_C], DT_MM, kind="ExternalInput").ap()
    bias = nc.dram_tensor("bias", [O_C], DT_F32, kind="ExternalInput").ap()
    out = nc.dram_tensor("out", [B_C, O_C], DT_F32, kind="ExternalOutput").ap()

    # 3D views: partition-major tiling of the contraction dim
    xT3 = xT.rearrange("(ko p) b -> p ko b", p=P)      # [128, 32, 2048]
    wT3 = wT.rearrange("(ko p) o -> p ko o", p=P)      # [128, 32, 1024]
    out3 = out.rearrange("(mo p) o -> p mo o", p=P)    # [128, 16, 1024]

    kchunk = KT // W_CHUNKS

    with tile.TileContext(nc) as tc:
        with tc.tile_pool(name="wres", bufs=1) as wres, \
             tc.tile_pool(name="bias_p", bufs=1) as bias_p, \
             tc.tile_pool(name="xin", bufs=2) as xin, \
             tc.tile_pool(name="outp", bufs=2) as outp, \
             tc.tile_pool(name="ps", bufs=1, space="PSUM") as ps:

            # Resident W^T: [128, 32, 1024] float32r = 128KB/partition
            w_sb = wres.tile([P, KT, O_C], DT_MM)
            bias_sb = bias_p.tile([P, O_C], DT_F32)

            def finish_m(m, psums):
                o_sb = outp.tile([P, O_C], DT_F32, tag="otile", name=f"o_{m}")
                for n in range(NT):
                    nsl = bass.ts(n, NFREE)
                    nc.vector.tensor_add(o_sb[:, nsl], psums[n][:],
                                         bias_sb[:, nsl])
                nc.sync.dma_start(out3[:, m, :], o_sb[:])

            def alloc_psums(m):
                return [ps.tile([P, NFREE], DT_F32, tag=f"ps{m % 4}_{n}",
                                name=f"psum_{m}_{n}")
                        for n in range(NT)]

            def mm(psums, xh, k):
                x_sb, off = xh
                for n in range(NT):
                    nc.tensor.matmul(
                        psums[n][:],
                        x_sb[:, k, off:off + P],
                        w_sb[:, k, bass.ts(n, NFREE)],
                        start=(k == 0),
                        stop=(k == KT - 1),
                    )

            # W chunks stream on the SP HWDGE ring; whole-x tiles ride
            # the ACT ring so the two streams share SDMA bandwidth without
            # queuing behind each other. Bias rides the SP ring *behind* W
            # (it is needed late; keeps the ACT ring clear for x tiles).
            def load_x2(mp):
                t = xin.tile([P, KT, 2 * P], DT_MM, tag="xtile",
                             name=f"x2_{mp}")
                nc.scalar.dma_start(t[:],
                                    xT3[:, :, bass.ds(mp * 2 * P, 2 * P)])
                return t

            pairs = [load_x2(mp) for mp in range(HEAD_M // 2)]
            head_x = [(pairs[m // 2], (m % 2) * P) for m in range(HEAD_M)]
            for j in range(W_CHUNKS):
                ksl = bass.ts(j, kchunk)
                nc.sync.dma_start(w_sb[:, ksl, :], wT3[:, ksl, :])
            nc.sync.dma_start(bias_sb[:], bias[None, :].to_broadcast((P, O_C)))

            # Phase 0: stripe the first HEAD_M m-tiles by W k-chunk so the
            # PE consumes each W chunk as it lands (HEAD_M*NT PSUM banks).
            head_ps = [alloc_psums(m) for m in range(HEAD_M)]
            for j in range(W_CHUNKS):
                for m in range(HEAD_M):
                    for kk in range(kchunk):
                        mm(head_ps[m], head_x[m], j * kchunk + kk)
            for m in range(HEAD_M):
                finish_m(m, head_ps[m])

            # Steady state: W fully resident; one m-tile at a time.
            pair_cache = {}
            for m in range(HEAD_M, MT):
                mp = m // 2
                if mp not in pair_cache:
                    pair_cache[mp] = load_x2(mp)
                psums = alloc_psums(m)
                for k in range(KT):
                    mm(psums, (pair_cache[mp], (m % 2) * P), k)
                finish_m(m, psums)

    nc.compile()
    return nc


_NC = None


def _get_nc():
    global _NC
    if _NC is None:
        _NC = _build_kernel()
    return _NC


def kernel(x: np.ndarray, weights: np.ndarray, bias: np.ndarray) -> np.ndarray:
    x = np.asarray(x, dtype=np.float32)
    weights = np.asarray(weights, dtype=np.float32)
    bias = np.asarray(bias, dtype=np.float32)
    assert x.shape == (BATCH, IN_F) and weights.shape == (OUT_F, IN_F)

    nc = _get_nc()

    in_maps = []
    for c in range(N_CORES):
        bi, oj = divmod(c, O_SHARDS)
        xs = slice(bi * B_C, (bi + 1) * B_C)
        os_ = slice(oj * O_C, (oj + 1) * O_C)
        in_maps.append({
            "xT": np.ascontiguousarray(x[xs, :].T),
            "wT": np.ascontiguousarray(weights[os_, :].T),
            "bias": np.ascontiguousarray(bias[os_]),
        })

    res = run_bass_kernel_spmd(nc, in_maps, core_ids=list(range(N_CORES)))

    out = np.empty((BATCH, OUT_F), dtype=np.float32)
    for c in range(N_CORES):
        bi, oj = divmod(c, O_SHARDS)
        out[bi * B_C:(bi + 1) * B_C, oj * O_C:(oj + 1) * O_C] = \
            res.results[c]["out"]
    return out

